# Initial kernel scaffold
#
"""DyadicQALoRA fused kernel for Trainium2 (8 NeuronCores).

Computes, for x:[B,S,Din], weight:[Dout,Din], bias:[Dout], lora_A:[Din,16],
lora_B:[16,Dout]:

    x_q, x_scale = per-token int8 absmax quant(x)        (exact RNE rounding)
    w_q, w_scale = ternary absmean quant(weight)
    a_q, a_s     = per-tensor int8 quant(lora_A)
    b_q, b_s     = per-tensor int8 quant(lora_B)
    out = (x_q @ w_q.T) * (w_scale*x_scale) + bias
        + ((x_q @ a_q) @ b_q) * (x_scale*a_s*b_s*2.0)

Sharding: 2-D tensor/data hybrid over 8 cores — 4 token groups x 2
out-feature groups.  The only collective is a 1-scalar AllReduce for the
global absmean weight scale (each core reduces a disjoint 1/8 row shard).

Device math notes:
  - x_q in [-127,127] and w_q in {-1,0,1} are exact in bf16 (and w_q in
    fp8e4), so the base matmul accumulating in fp32 PSUM is bit-exact.
  - round-to-nearest-even is done with the fp32 magic constant 1.5*2^23.
  - LoRA path: xa^T = a_q^T-as-rhs reusing the base stationary tiles;
    xa*c2 (c2 = a_s*b_s*2/w_scale) is split into two bf16 pieces whose
    matmuls accumulate onto the base PSUM, so the epilogue is just
    out = psum * (x_scale*w_scale) + bias.
"""

import os
import sys
import functools

import numpy as np

for _p in ("/opt/trn_rl_repo", "/root/.axon_site/_ro/trn_rl_repo"):
    if os.path.isdir(_p) and _p not in sys.path:
        sys.path.insert(0, _p)

import ml_dtypes  # noqa: E402
import concourse.bass as bass  # noqa: E402
import concourse.mybir as mybir  # noqa: E402
from concourse import bacc  # noqa: E402
from concourse import bass_isa  # noqa: E402
from concourse import tile  # noqa: E402

F32 = mybir.dt.float32
BF16 = mybir.dt.bfloat16
FP8 = mybir.dt.float8e4

MAGIC = 12582912.0  # 1.5 * 2**23 : fp32 add/sub gives exact RNE round
QMAX = 127.0
EPS = 1e-6
SCALING = 2.0  # alpha/rank = 32/16
N_CORES = 8
R_TOK = 4  # token groups
C_OUT = 2  # out-feature groups


def build_nc(TOK, DIN, DOUT_C, WSC_ROWS, N_FULL_W, RANK=16, use_fp8_w=True):
    """Build the per-core (SPMD) Bass program.

    TOK: tokens per core; DIN: contraction dim; DOUT_C: out features per
    core; WSC_ROWS: rows of the weight-scale shard (full_rows/8);
    N_FULL_W: element count of the FULL weight (mean divisor).
    """
    assert TOK % 128 == 0 and DIN % 128 == 0 and DOUT_C % 512 == 0
    KT = DIN // 128
    MT = TOK // 128
    NCH = DOUT_C // 512
    WT = DOUT_C // 128
    WSCT = WSC_ROWS // 128
    W_DT = FP8 if use_fp8_w else BF16

    nc = bacc.Bacc(
        "TRN2", target_bir_lowering=False, debug=False, num_devices=N_CORES,
    )

    x_in = nc.dram_tensor("x_in", [TOK, DIN], F32, kind="ExternalInput")
    w_in = nc.dram_tensor("w_in", [DOUT_C, DIN], F32, kind="ExternalInput")
    wsc_in = nc.dram_tensor("wsc_in", [WSC_ROWS, DIN], F32, kind="ExternalInput")
    bias_in = nc.dram_tensor("bias_in", [1, DOUT_C], F32, kind="ExternalInput")
    la_in = nc.dram_tensor("la_in", [DIN, RANK], F32, kind="ExternalInput")
    lbf_in = nc.dram_tensor("lbf_in", [16, N_FULL_W // DIN], F32, kind="ExternalInput")
    lb_in = nc.dram_tensor("lb_in", [16, DOUT_C], F32, kind="ExternalInput")
    ident_in = nc.dram_tensor("ident_in", [128, 128], BF16, kind="ExternalInput")
    out_d = nc.dram_tensor("out", [TOK, DOUT_C], F32, kind="ExternalOutput")

    DOUT_FULL = N_FULL_W // DIN

    def newton_recip(pool, a, tag):
        """IEEE 1/x (trn2 Reciprocal is exactly rounded; verified bitwise)."""
        P = a.shape[0]
        r0 = pool.tile([P, 1], F32, tag=tag + "_r0", name=tag + "_r0")
        nc.vector.reciprocal(r0[:], a[:])
        return r0

    with tile.TileContext(nc) as tc:
        with (
            tc.tile_pool(name="const", bufs=1) as cpool,
            tc.tile_pool(name="wqres", bufs=1) as wqpool,
            tc.tile_pool(name="bigstage", bufs=2) as bpool,
            tc.tile_pool(name="xstage", bufs=2) as xpool,
            tc.tile_pool(name="small", bufs=2) as spool,
            tc.tile_pool(name="psum", bufs=1, space="PSUM") as ppool,
            tc.tile_pool(name="psum2", bufs=2, space="PSUM") as p2pool,
            tc.tile_pool(name="dram", bufs=1, space="DRAM") as dpool,
        ):
            # ---------------- constants / prep ----------------
            ident = cpool.tile([128, 128], BF16, tag="ident")
            nc.sync.dma_start(ident[:], ident_in[:])

            # ---- global |w| mean -> w_scale (AllReduce over 8 cores) ----
            wsums = cpool.tile([128, WSCT], F32, tag="wsums")
            for t in range(WSCT):
                wst = bpool.tile([128, DIN], F32, tag="wtile")
                nc.gpsimd.dma_start(wst[:], wsc_in[t * 128:(t + 1) * 128, :])
                nc.vector.tensor_reduce(
                    wsums[:, t:t + 1], wst[:], axis=mybir.AxisListType.X,
                    op=mybir.AluOpType.add, apply_absolute_value=True,
                )
            wsum_p = cpool.tile([128, 1], F32, tag="wsum_p")
            nc.vector.tensor_reduce(
                wsum_p[:], wsums[:], axis=mybir.AxisListType.X,
                op=mybir.AluOpType.add,
            )
            wsum_b = cpool.tile([128, 1], F32, tag="wsum_b")
            nc.gpsimd.partition_all_reduce(
                wsum_b[:], wsum_p[:], channels=128,
                reduce_op=bass_isa.ReduceOp.add,
            )
            cc_in = dpool.tile([1, 1], F32)
            cc_out = dpool.tile([1, 1], F32)
            nc.sync.dma_start(cc_in[:], wsum_b[0:1, :])
            nc.gpsimd.collective_compute(
                "AllReduce", mybir.AluOpType.add,
                replica_groups=[list(range(N_CORES))],
                ins=[cc_in.opt()], outs=[cc_out.opt()],
            )
            wsg = cpool.tile([1, 1], F32, tag="wsg")
            nc.sync.dma_start(wsg[:], cc_out[:])
            wsg_b = cpool.tile([128, 1], F32, tag="wsg_b")
            nc.gpsimd.partition_broadcast(wsg_b[:], wsg[:])
            ws_t = cpool.tile([128, 1], F32, tag="ws_t")
            # mean = sum / N (N power of two -> exact), clip at EPS
            nc.vector.tensor_scalar(
                ws_t[:], wsg_b[:], 1.0 / float(N_FULL_W), EPS,
                op0=mybir.AluOpType.mult, op1=mybir.AluOpType.max,
            )
            inv_ws = newton_recip(cpool, ws_t, "inv_ws")

            bias_b = cpool.tile([128, DOUT_C], F32, tag="bias_b")
            bias_row = bpool.tile([1, DOUT_C], F32, tag="wtile", name="bias_row")
            nc.sync.dma_start(bias_row[:], bias_in[:])
            nc.gpsimd.partition_broadcast(bias_b[:], bias_row[:])


            # ---------------- lora_A quant ----------------
            la_s = bpool.tile([128, KT, RANK], F32, tag="wtile", name="la_s")
            nc.sync.dma_start(
                la_s[:], la_in.rearrange("(kt p) r -> p kt r", p=128)
            )
            amax0 = cpool.tile([128, 1], F32, tag="amax0")
            nc.vector.tensor_reduce(
                amax0[:], la_s[:], axis=mybir.AxisListType.XY,
                op=mybir.AluOpType.max, apply_absolute_value=True,
            )
            amax = cpool.tile([128, 1], F32, tag="amax")
            nc.gpsimd.partition_all_reduce(
                amax[:], amax0[:], channels=128, reduce_op=bass_isa.ReduceOp.max,
            )
            amax_c = cpool.tile([128, 1], F32, tag="amax_c")
            nc.vector.tensor_scalar(
                amax_c[:], amax[:], EPS, None, op0=mybir.AluOpType.max,
            )
            ia = newton_recip(cpool, amax_c, "ia")
            ia127 = cpool.tile([128, 1], F32, tag="ia127")
            nc.vector.tensor_scalar(
                ia127[:], ia[:], QMAX, None, op0=mybir.AluOpType.mult,
            )
            a_sc = cpool.tile([128, 1], F32, tag="a_sc")  # a_s = amax/127
            nc.vector.tensor_scalar(
                a_sc[:], amax_c[:], 1.0 / QMAX, None, op0=mybir.AluOpType.mult,
            )
            nc.vector.tensor_scalar(
                la_s[:], la_s[:], ia127[:], MAGIC,
                op0=mybir.AluOpType.mult, op1=mybir.AluOpType.add,
            )
            a_q = cpool.tile([128, KT, RANK], BF16, tag="a_q")
            nc.scalar.activation(
                a_q[:], la_s[:], mybir.ActivationFunctionType.Copy, bias=-MAGIC,
            )

            # ---------------- lora_B quant ----------------
            lbf_s = bpool.tile([16, DOUT_FULL], F32, tag="wtile", name="lbf_s")
            nc.sync.dma_start(lbf_s[:], lbf_in[:])
            bmax0 = cpool.tile([16, 1], F32, tag="bmax0")
            nc.vector.tensor_reduce(
                bmax0[:], lbf_s[:], axis=mybir.AxisListType.X,
                op=mybir.AluOpType.max, apply_absolute_value=True,
            )
            bmax = cpool.tile([16, 1], F32, tag="bmax")
            nc.gpsimd.partition_all_reduce(
                bmax[:], bmax0[:], channels=16, reduce_op=bass_isa.ReduceOp.max,
            )
            bmax_c = cpool.tile([16, 1], F32, tag="bmax_c")
            nc.vector.tensor_scalar(
                bmax_c[:], bmax[:], EPS, None, op0=mybir.AluOpType.max,
            )
            ib = newton_recip(cpool, bmax_c, "ib")
            ib127 = cpool.tile([16, 1], F32, tag="ib127")
            nc.vector.tensor_scalar(
                ib127[:], ib[:], QMAX, None, op0=mybir.AluOpType.mult,
            )
            lb_s = bpool.tile([16, DOUT_C], F32, tag="wtile", name="lb_s")
            nc.sync.dma_start(lb_s[:], lb_in[:])
            nc.vector.tensor_scalar(
                lb_s[:], lb_s[:], ib127[:], MAGIC,
                op0=mybir.AluOpType.mult, op1=mybir.AluOpType.add,
            )
            b_q = cpool.tile([16, DOUT_C], BF16, tag="b_q")
            nc.scalar.activation(
                b_q[:], lb_s[:], mybir.ActivationFunctionType.Copy, bias=-MAGIC,
            )

            # c2 = a_s * b_s * SCALING / w_scale  (on 128 partitions)
            bmax_b = cpool.tile([128, 1], F32, tag="bmax_b")
            nc.gpsimd.partition_broadcast(bmax_b[:], bmax_c[0:1, :])
            b_sc = cpool.tile([128, 1], F32, tag="b_sc")
            nc.vector.tensor_scalar(
                b_sc[:], bmax_b[:], 1.0 / QMAX, None, op0=mybir.AluOpType.mult,
            )
            c2a = cpool.tile([128, 1], F32, tag="c2a")
            nc.vector.tensor_tensor(
                c2a[:], a_sc[:], b_sc[:], op=mybir.AluOpType.mult,
            )
            c2b = cpool.tile([128, 1], F32, tag="c2b")
            nc.vector.tensor_scalar(
                c2b[:], c2a[:], SCALING, None, op0=mybir.AluOpType.mult,
            )
            c2 = cpool.tile([128, 1], F32, tag="c2")
            nc.vector.tensor_tensor(
                c2[:], c2b[:], inv_ws[:], op=mybir.AluOpType.mult,
            )

            # ---------------- weight quant + transpose ----------------
            # w_qT chunks: [d(128), k-tile, 512 douts] ; chunk c covers
            # douts [512c, 512c+512).
            wqT = [
                wqpool.tile([128, KT, 512], W_DT, tag=f"wqT{c}", name=f"wqT{c}")
                for c in range(NCH)
            ]
            for j in range(WT):
                wt = bpool.tile([128, DIN], F32, tag="wtile")
                nc.gpsimd.dma_start(wt[:], w_in[j * 128:(j + 1) * 128, :])
                wt2 = bpool.tile([128, DIN], F32, tag="wt2", bufs=1)
                nc.vector.tensor_scalar(
                    wt2[:], wt[:], inv_ws[:], 1.49,
                    op0=mybir.AluOpType.mult, op1=mybir.AluOpType.min,
                )
                nc.vector.tensor_scalar(
                    wt[:], wt2[:], -1.49, MAGIC,
                    op0=mybir.AluOpType.max, op1=mybir.AluOpType.add,
                )
                wq_t = bpool.tile([128, DIN], BF16, tag="wq_t")
                nc.scalar.activation(
                    wq_t[:], wt[:], mybir.ActivationFunctionType.Copy, bias=-MAGIC,
                )
                wqT_st = bpool.tile([128, KT, 128], BF16, tag="wqT_st", bufs=1)
                nc.sync.dma_start(wqT_st[:], wq_t[:], transpose=True)
                c, sl = j // 4, j % 4
                nc.scalar.copy(
                    wqT[c][:, :, sl * 128:(sl + 1) * 128], wqT_st[:],
                )

            # ---------------- main loop over token tiles ----------------
            for m in range(MT):
                xt = xpool.tile([128, DIN], F32, tag="xt", bufs=1)
                nc.gpsimd.dma_start(xt[:], x_in[m * 128:(m + 1) * 128, :])
                sx = spool.tile([128, 1], F32, tag="sx")
                nc.vector.tensor_reduce(
                    sx[:], xt[:], axis=mybir.AxisListType.X,
                    op=mybir.AluOpType.max, apply_absolute_value=True,
                )
                sxc = spool.tile([128, 1], F32, tag="sxc")
                nc.vector.tensor_scalar(
                    sxc[:], sx[:], EPS, None, op0=mybir.AluOpType.max,
                )
                xs_t = spool.tile([128, 1], F32, tag="xs_t")  # x_scale
                nc.vector.tensor_scalar(
                    xs_t[:], sxc[:], 1.0 / QMAX, None, op0=mybir.AluOpType.mult,
                )
                ix = newton_recip(spool, xs_t, "ix")
                xsws = spool.tile([128, 1], F32, tag="xsws")
                nc.vector.tensor_tensor(
                    xsws[:], xs_t[:], ws_t[:], op=mybir.AluOpType.mult,
                )
                xt2 = bpool.tile([128, DIN], F32, tag="wt2", name="xt2", bufs=1)
                nc.scalar.activation(
                    xt2[:], xt[:], mybir.ActivationFunctionType.Copy,
                    bias=MAGIC, scale=ix[:],
                )
                xq_t = xpool.tile([128, DIN], BF16, tag="xq_t", bufs=1)
                nc.scalar.activation(
                    xq_t[:], xt2[:], mybir.ActivationFunctionType.Copy, bias=-MAGIC,
                )
                xqT = xpool.tile([128, KT, 128], BF16, tag="xqT")
                nc.sync.dma_start(xqT[:], xq_t[:], transpose=True)

                # ---- matmuls ----
                psum_b = ppool.tile([128, DOUT_C], F32, tag="psum_b")
                psum_xa = p2pool.tile([128, RANK], F32, tag="psum_xa")
                for k in range(KT):
                    lhs = xqT[:, k, :]
                    for c in range(NCH):
                        nc.tensor.matmul(
                            psum_b[:, c * 512:(c + 1) * 512],
                            lhs, wqT[c][:, k, :],
                            start=(k == 0), stop=False,
                        )
                    nc.tensor.matmul(
                        psum_xa[:], lhs, a_q[:, k, :],
                        start=(k == 0), stop=(k == KT - 1),
                    )

                # ---- lora second stage: split xa*c2 into 2 bf16 pieces ----
                v_xa = spool.tile([128, RANK], F32, tag="v_xa")
                nc.vector.tensor_scalar(
                    v_xa[:], psum_xa[:], c2[:], None, op0=mybir.AluOpType.mult,
                )
                # hi at cols 0:16, lo at cols 32:48 -> after transpose the
                # pieces sit at 32-aligned base partitions (BIR requires
                # partition access to start at 0/32/64/96).
                pieces = spool.tile([128, 4 * RANK], BF16, tag="pieces")
                nc.vector.tensor_copy(pieces[:, 0:RANK], v_xa[:])
                hi_f = spool.tile([128, RANK], F32, tag="hi_f")
                nc.vector.tensor_copy(hi_f[:], pieces[:, 0:RANK])
                nc.vector.tensor_tensor(
                    pieces[:, 2 * RANK:3 * RANK], v_xa[:], hi_f[:],
                    op=mybir.AluOpType.subtract,
                )
                piecesT_ps = p2pool.tile([4 * RANK, 128], BF16, tag="piecesT_ps")
                nc.tensor.transpose(piecesT_ps[:], pieces[:], ident[:])
                # each piece copied to a base-0 tile (PE needs matching
                # base partitions for lhsT and rhs)
                piecesT = [
                    spool.tile([RANK, 128], BF16, tag=f"piecesT{p}",
                               name=f"piecesT{p}")
                    for p in range(2)
                ]
                for p in range(2):
                    nc.scalar.copy(
                        piecesT[p][:],
                        piecesT_ps[2 * p * RANK:(2 * p + 1) * RANK, :])
                for p in range(2):
                    lhs_p = piecesT[p][:]
                    for c in range(NCH):
                        nc.tensor.matmul(
                            psum_b[:, c * 512:(c + 1) * 512],
                            lhs_p, b_q[:, c * 512:(c + 1) * 512],
                            start=False, stop=(p == 1),
                        )

                # ---- epilogue: out = psum * (x_scale*w_scale) + bias ----
                u = xpool.tile([128, DOUT_C], F32, tag="u", bufs=1)
                nc.scalar.activation(
                    u[:], psum_b[:], mybir.ActivationFunctionType.Copy,
                    bias=0.0, scale=xsws[:],
                )
                nc.vector.tensor_tensor(
                    u[:], u[:], bias_b[:], op=mybir.AluOpType.add,
                )
                nc.scalar.dma_start(out_d[m * 128:(m + 1) * 128, :], u[:])

    nc.compile()
    return nc


# ----------------------------------------------------------------------
# host-side wrapper
# ----------------------------------------------------------------------

@functools.lru_cache(maxsize=2)
def _get_nc(TOK, DIN, DOUT_C, WSC_ROWS, N_FULL_W):
    return build_nc(TOK, DIN, DOUT_C, WSC_ROWS, N_FULL_W)


def _prep(x, weight, bias, lora_A, lora_B):
    B, S, DIN = x.shape
    DOUT = weight.shape[0]
    NTOK = B * S
    assert NTOK % R_TOK == 0 and DOUT % C_OUT == 0 and DOUT % N_CORES == 0
    TOK = NTOK // R_TOK
    DOUT_C = DOUT // C_OUT
    WSC_ROWS = DOUT // N_CORES
    N_FULL_W = DOUT * DIN

    nc = _get_nc(TOK, DIN, DOUT_C, WSC_ROWS, N_FULL_W)

    x2 = np.ascontiguousarray(x.reshape(NTOK, DIN).astype(np.float32, copy=False))
    weight = np.ascontiguousarray(weight.astype(np.float32, copy=False))
    ident = np.eye(128, dtype=ml_dtypes.bfloat16)

    in_maps = []
    for core in range(N_CORES):
        i, j = core // C_OUT, core % C_OUT
        in_maps.append({
            "x_in": np.ascontiguousarray(x2[i * TOK:(i + 1) * TOK]),
            "w_in": np.ascontiguousarray(weight[j * DOUT_C:(j + 1) * DOUT_C]),
            "wsc_in": np.ascontiguousarray(
                weight[core * WSC_ROWS:(core + 1) * WSC_ROWS]),
            "bias_in": np.ascontiguousarray(
                bias[j * DOUT_C:(j + 1) * DOUT_C].reshape(1, DOUT_C)),
            "la_in": np.ascontiguousarray(lora_A.astype(np.float32, copy=False)),
            "lbf_in": np.ascontiguousarray(lora_B.astype(np.float32, copy=False)),
            "lb_in": np.ascontiguousarray(lora_B[:, j * DOUT_C:(j + 1) * DOUT_C]),
            "ident_in": ident,
        })
    return nc, in_maps, (B, S, NTOK, TOK, DOUT, DOUT_C)


def kernel(x, weight, bias, lora_A, lora_B):
    from concourse.bass_utils import run_bass_kernel_spmd

    nc, in_maps, (B, S, NTOK, TOK, DOUT, DOUT_C) = _prep(
        x, weight, bias, lora_A, lora_B)
    res = run_bass_kernel_spmd(nc, in_maps, core_ids=list(range(N_CORES)))

    out = np.empty((NTOK, DOUT), np.float32)
    for core in range(N_CORES):
        i, j = core // C_OUT, core % C_OUT
        out[i * TOK:(i + 1) * TOK, j * DOUT_C:(j + 1) * DOUT_C] = \
            res.results[core]["out"]
    return out.reshape(B, S, DOUT)


def _install_profile_shim():
    """Register the axon NTFF profile hook (antenv.axon_hooks is absent in
    this image; libaxon_pjrt.so supports the profile C ABI directly) and
    stub out the network-dependent artifact upload."""
    import types
    import ctypes
    import contextlib

    try:
        import antenv.axon_hooks  # noqa: F401
        have = True
    except ImportError:
        have = False
    if not have:
        so = "/opt/axon/libaxon_pjrt.so"
        lib = ctypes.CDLL(so)
        lib.axon_start_nrt_profile.argtypes = [
            ctypes.POINTER(ctypes.c_int64), ctypes.c_size_t]
        lib.axon_start_nrt_profile.restype = ctypes.c_int64
        lib.axon_stop_nrt_profile.argtypes = [ctypes.c_char_p]
        lib.axon_stop_nrt_profile.restype = ctypes.c_int64

        @contextlib.contextmanager
        def _hook(output_dir, device_ids):
            import jax
            jax.devices()
            if device_ids:
                ids = (ctypes.c_int64 * len(device_ids))(*device_ids)
                rc = lib.axon_start_nrt_profile(ids, len(device_ids))
            else:
                rc = lib.axon_start_nrt_profile(None, 0)
            if rc != 0:
                raise RuntimeError(f"axon_start_nrt_profile rc={rc}")
            try:
                yield
            finally:
                lib.axon_stop_nrt_profile(str(output_dir).encode())

        import antenv
        mod = types.ModuleType("antenv.axon_hooks")
        mod.get_axon_ntff_profile_hook = lambda: _hook
        mod.set_axon_ntff_profile_hook = lambda h: None
        sys.modules["antenv.axon_hooks"] = mod
        antenv.axon_hooks = mod

    from concourse import bass_utils
    bass_utils.upload_artifacts = lambda tmpdir: f"local:{tmpdir}"


def timed_run(inputs, trace_cores=None):
    """Run with NTFF tracing; returns max exec_time_ns across traced cores."""
    import tempfile
    _install_profile_shim()
    from concourse.bass_utils import run_bass_kernel_spmd

    nc, in_maps, _ = _prep(**inputs)
    res = run_bass_kernel_spmd(
        nc, in_maps, core_ids=list(range(N_CORES)), trace=True,
        trace_cores=trace_cores if trace_cores is not None
        else list(range(N_CORES)),
        tmpdir=tempfile.mkdtemp(prefix="dyadic_trace_"),
    )
    return res.exec_time_ns



# revision 1
# speedup vs baseline: 1.0343x; 1.0343x over previous
"""DyadicQALoRA fused kernel for Trainium2 (8 NeuronCores).

Computes, for x:[B,S,Din], weight:[Dout,Din], bias:[Dout], lora_A:[Din,16],
lora_B:[16,Dout]:

    x_q, x_scale = per-token int8 absmax quant(x)        (exact RNE rounding)
    w_q, w_scale = ternary absmean quant(weight)
    a_q, a_s     = per-tensor int8 quant(lora_A)
    b_q, b_s     = per-tensor int8 quant(lora_B)
    out = (x_q @ w_q.T) * (w_scale*x_scale) + bias
        + ((x_q @ a_q) @ b_q) * (x_scale*a_s*b_s*2.0)

Sharding: 2-D tensor/data hybrid over 8 cores — 4 token groups x 2
out-feature groups.  The only collective is a 1-scalar AllReduce for the
global absmean weight scale (each core reduces a disjoint 1/8 row shard).

Device math notes:
  - x_q in [-127,127] and w_q in {-1,0,1} are exact in bf16 (and w_q in
    fp8e4), so the base matmul accumulating in fp32 PSUM is bit-exact.
  - round-to-nearest-even is done with the fp32 magic constant 1.5*2^23.
  - LoRA path: xa^T = a_q^T-as-rhs reusing the base stationary tiles;
    xa*c2 (c2 = a_s*b_s*2/w_scale) is split into two bf16 pieces whose
    matmuls accumulate onto the base PSUM, so the epilogue is just
    out = psum * (x_scale*w_scale) + bias.
"""

import os
import sys
import functools

import numpy as np

for _p in ("/opt/trn_rl_repo", "/root/.axon_site/_ro/trn_rl_repo"):
    if os.path.isdir(_p) and _p not in sys.path:
        sys.path.insert(0, _p)

import ml_dtypes  # noqa: E402
import concourse.bass as bass  # noqa: E402
import concourse.mybir as mybir  # noqa: E402
from concourse import bacc  # noqa: E402
from concourse import bass_isa  # noqa: E402
from concourse import tile  # noqa: E402

F32 = mybir.dt.float32
BF16 = mybir.dt.bfloat16
FP8 = mybir.dt.float8e4

MAGIC = 12582912.0  # 1.5 * 2**23 : fp32 add/sub gives exact RNE round
QMAX = 127.0
EPS = 1e-6
SCALING = 2.0  # alpha/rank = 32/16
N_CORES = 8
R_TOK = 4  # token groups
C_OUT = 2  # out-feature groups


def build_nc(TOK, DIN, DOUT_C, WSC_ROWS, N_FULL_W, RANK=16, use_fp8_w=True):
    """Build the per-core (SPMD) Bass program.

    TOK: tokens per core; DIN: contraction dim; DOUT_C: out features per
    core; WSC_ROWS: rows of the weight-scale shard (full_rows/8);
    N_FULL_W: element count of the FULL weight (mean divisor).
    """
    assert TOK % 128 == 0 and DIN % 128 == 0 and DOUT_C % 512 == 0
    KT = DIN // 128
    MT = TOK // 128
    NCH = DOUT_C // 512
    WT = DOUT_C // 128
    WSCT = WSC_ROWS // 128
    W_DT = FP8 if use_fp8_w else BF16

    nc = bacc.Bacc(
        "TRN2", target_bir_lowering=False, debug=False, num_devices=N_CORES,
    )

    x_in = nc.dram_tensor("x_in", [TOK, DIN], F32, kind="ExternalInput")
    w_in = nc.dram_tensor("w_in", [DOUT_C, DIN], F32, kind="ExternalInput")
    wsc_in = nc.dram_tensor("wsc_in", [WSC_ROWS, DIN], F32, kind="ExternalInput")
    bias_in = nc.dram_tensor("bias_in", [1, DOUT_C], F32, kind="ExternalInput")
    la_in = nc.dram_tensor("la_in", [DIN, RANK], F32, kind="ExternalInput")
    lbf_in = nc.dram_tensor("lbf_in", [16, N_FULL_W // DIN], F32, kind="ExternalInput")
    lb_in = nc.dram_tensor("lb_in", [16, DOUT_C], F32, kind="ExternalInput")
    ident_in = nc.dram_tensor("ident_in", [128, 128], BF16, kind="ExternalInput")
    out_d = nc.dram_tensor("out", [TOK, DOUT_C], F32, kind="ExternalOutput")

    DOUT_FULL = N_FULL_W // DIN

    def newton_recip(pool, a, tag):
        """IEEE 1/x (trn2 Reciprocal is exactly rounded; verified bitwise)."""
        P = a.shape[0]
        r0 = pool.tile([P, 1], F32, tag=tag + "_r0", name=tag + "_r0")
        nc.vector.reciprocal(r0[:], a[:])
        return r0

    with tile.TileContext(nc) as tc:
        with (
            tc.tile_pool(name="const", bufs=1) as cpool,
            tc.tile_pool(name="wqres", bufs=1) as wqpool,
            tc.tile_pool(name="bigstage", bufs=2) as bpool,
            tc.tile_pool(name="xstage", bufs=2) as xpool,
            tc.tile_pool(name="small", bufs=2) as spool,
            tc.tile_pool(name="psum", bufs=1, space="PSUM") as ppool,
            tc.tile_pool(name="psum2", bufs=2, space="PSUM") as p2pool,
            tc.tile_pool(name="dram", bufs=1, space="DRAM") as dpool,
        ):
            # ---------------- constants / prep ----------------
            ident = cpool.tile([128, 128], BF16, tag="ident")
            nc.sync.dma_start(ident[:], ident_in[:])

            # ---- global |w| mean -> w_scale (AllReduce over 8 cores) ----
            wsums = cpool.tile([128, WSCT], F32, tag="wsums")
            for t in range(WSCT):
                wst = bpool.tile([128, DIN], F32, tag="wtile")
                nc.gpsimd.dma_start(wst[:], wsc_in[t * 128:(t + 1) * 128, :])
                nc.vector.tensor_reduce(
                    wsums[:, t:t + 1], wst[:], axis=mybir.AxisListType.X,
                    op=mybir.AluOpType.add, apply_absolute_value=True,
                )
            wsum_p = cpool.tile([128, 1], F32, tag="wsum_p")
            nc.vector.tensor_reduce(
                wsum_p[:], wsums[:], axis=mybir.AxisListType.X,
                op=mybir.AluOpType.add,
            )
            wsum_b = cpool.tile([128, 1], F32, tag="wsum_b")
            nc.gpsimd.partition_all_reduce(
                wsum_b[:], wsum_p[:], channels=128,
                reduce_op=bass_isa.ReduceOp.add,
            )
            cc_in = dpool.tile([1, 1], F32)
            cc_out = dpool.tile([1, 1], F32)
            nc.sync.dma_start(cc_in[:], wsum_b[0:1, :])
            nc.gpsimd.collective_compute(
                "AllReduce", mybir.AluOpType.add,
                replica_groups=[list(range(N_CORES))],
                ins=[cc_in.opt()], outs=[cc_out.opt()],
            )
            wsg = cpool.tile([1, 1], F32, tag="wsg")
            nc.sync.dma_start(wsg[:], cc_out[:])
            wsg_b = cpool.tile([128, 1], F32, tag="wsg_b")
            nc.gpsimd.partition_broadcast(wsg_b[:], wsg[:])
            ws_t = cpool.tile([128, 1], F32, tag="ws_t")
            # mean = sum / N (N power of two -> exact), clip at EPS
            nc.vector.tensor_scalar(
                ws_t[:], wsg_b[:], 1.0 / float(N_FULL_W), EPS,
                op0=mybir.AluOpType.mult, op1=mybir.AluOpType.max,
            )
            inv_ws = newton_recip(cpool, ws_t, "inv_ws")

            bias_b = cpool.tile([128, DOUT_C], F32, tag="bias_b")
            bias_row = bpool.tile([1, DOUT_C], F32, tag="wtile", name="bias_row")
            nc.sync.dma_start(bias_row[:], bias_in[:])
            nc.gpsimd.partition_broadcast(bias_b[:], bias_row[:])


            # ---------------- lora_A quant ----------------
            la_s = bpool.tile([128, KT, RANK], F32, tag="wtile", name="la_s")
            nc.sync.dma_start(
                la_s[:], la_in.rearrange("(kt p) r -> p kt r", p=128)
            )
            amax0 = cpool.tile([128, 1], F32, tag="amax0")
            nc.vector.tensor_reduce(
                amax0[:], la_s[:], axis=mybir.AxisListType.XY,
                op=mybir.AluOpType.max, apply_absolute_value=True,
            )
            amax = cpool.tile([128, 1], F32, tag="amax")
            nc.gpsimd.partition_all_reduce(
                amax[:], amax0[:], channels=128, reduce_op=bass_isa.ReduceOp.max,
            )
            amax_c = cpool.tile([128, 1], F32, tag="amax_c")
            nc.vector.tensor_scalar(
                amax_c[:], amax[:], EPS, None, op0=mybir.AluOpType.max,
            )
            ia = newton_recip(cpool, amax_c, "ia")
            ia127 = cpool.tile([128, 1], F32, tag="ia127")
            nc.vector.tensor_scalar(
                ia127[:], ia[:], QMAX, None, op0=mybir.AluOpType.mult,
            )
            a_sc = cpool.tile([128, 1], F32, tag="a_sc")  # a_s = amax/127
            nc.vector.tensor_scalar(
                a_sc[:], amax_c[:], 1.0 / QMAX, None, op0=mybir.AluOpType.mult,
            )
            nc.vector.tensor_scalar(
                la_s[:], la_s[:], ia127[:], MAGIC,
                op0=mybir.AluOpType.mult, op1=mybir.AluOpType.add,
            )
            a_q = cpool.tile([128, KT, RANK], BF16, tag="a_q")
            nc.scalar.activation(
                a_q[:], la_s[:], mybir.ActivationFunctionType.Copy, bias=-MAGIC,
            )

            # ---------------- lora_B quant ----------------
            lbf_s = bpool.tile([16, DOUT_FULL], F32, tag="wtile", name="lbf_s")
            nc.sync.dma_start(lbf_s[:], lbf_in[:])
            bmax0 = cpool.tile([16, 1], F32, tag="bmax0")
            nc.vector.tensor_reduce(
                bmax0[:], lbf_s[:], axis=mybir.AxisListType.X,
                op=mybir.AluOpType.max, apply_absolute_value=True,
            )
            bmax = cpool.tile([16, 1], F32, tag="bmax")
            nc.gpsimd.partition_all_reduce(
                bmax[:], bmax0[:], channels=16, reduce_op=bass_isa.ReduceOp.max,
            )
            bmax_c = cpool.tile([16, 1], F32, tag="bmax_c")
            nc.vector.tensor_scalar(
                bmax_c[:], bmax[:], EPS, None, op0=mybir.AluOpType.max,
            )
            ib = newton_recip(cpool, bmax_c, "ib")
            ib127 = cpool.tile([16, 1], F32, tag="ib127")
            nc.vector.tensor_scalar(
                ib127[:], ib[:], QMAX, None, op0=mybir.AluOpType.mult,
            )
            lb_s = bpool.tile([16, DOUT_C], F32, tag="wtile", name="lb_s")
            nc.sync.dma_start(lb_s[:], lb_in[:])
            nc.vector.tensor_scalar(
                lb_s[:], lb_s[:], ib127[:], MAGIC,
                op0=mybir.AluOpType.mult, op1=mybir.AluOpType.add,
            )
            b_q = cpool.tile([16, DOUT_C], BF16, tag="b_q")
            nc.scalar.activation(
                b_q[:], lb_s[:], mybir.ActivationFunctionType.Copy, bias=-MAGIC,
            )

            # c2 = a_s * b_s * SCALING / w_scale  (on 128 partitions)
            bmax_b = cpool.tile([128, 1], F32, tag="bmax_b")
            nc.gpsimd.partition_broadcast(bmax_b[:], bmax_c[0:1, :])
            b_sc = cpool.tile([128, 1], F32, tag="b_sc")
            nc.vector.tensor_scalar(
                b_sc[:], bmax_b[:], 1.0 / QMAX, None, op0=mybir.AluOpType.mult,
            )
            c2a = cpool.tile([128, 1], F32, tag="c2a")
            nc.vector.tensor_tensor(
                c2a[:], a_sc[:], b_sc[:], op=mybir.AluOpType.mult,
            )
            c2b = cpool.tile([128, 1], F32, tag="c2b")
            nc.vector.tensor_scalar(
                c2b[:], c2a[:], SCALING, None, op0=mybir.AluOpType.mult,
            )
            c2 = cpool.tile([128, 1], F32, tag="c2")
            nc.vector.tensor_tensor(
                c2[:], c2b[:], inv_ws[:], op=mybir.AluOpType.mult,
            )

            # ---------------- weight quant + transpose ----------------
            # w_qT chunks: [d(128), k-tile, 512 douts] ; chunk c covers
            # douts [512c, 512c+512).
            wqT = [
                wqpool.tile([128, KT, 512], W_DT, tag=f"wqT{c}", name=f"wqT{c}")
                for c in range(NCH)
            ]
            for j in range(WT):
                wt = bpool.tile([128, DIN], F32, tag="wtile")
                nc.gpsimd.dma_start(wt[:], w_in[j * 128:(j + 1) * 128, :])
                wt2 = bpool.tile([128, DIN], F32, tag="wt2", bufs=1)
                nc.vector.tensor_scalar(
                    wt2[:], wt[:], inv_ws[:], 1.49,
                    op0=mybir.AluOpType.mult, op1=mybir.AluOpType.min,
                )
                nc.vector.tensor_scalar(
                    wt[:], wt2[:], -1.49, MAGIC,
                    op0=mybir.AluOpType.max, op1=mybir.AluOpType.add,
                )
                wq_t = bpool.tile([128, DIN], BF16, tag="wq_t")
                nc.scalar.activation(
                    wq_t[:], wt[:], mybir.ActivationFunctionType.Copy, bias=-MAGIC,
                )
                wqT_st = bpool.tile([128, KT, 128], BF16, tag="wqT_st", bufs=1)
                nc.sync.dma_start(wqT_st[:], wq_t[:], transpose=True)
                c, sl = j // 4, j % 4
                nc.scalar.copy(
                    wqT[c][:, :, sl * 128:(sl + 1) * 128], wqT_st[:],
                )

            # ---------------- main loop over token tiles ----------------
            for m in range(MT):
                xt = xpool.tile([128, DIN], F32, tag="xt", bufs=1)
                nc.gpsimd.dma_start(xt[:], x_in[m * 128:(m + 1) * 128, :])
                sx = spool.tile([128, 1], F32, tag="sx")
                nc.vector.tensor_reduce(
                    sx[:], xt[:], axis=mybir.AxisListType.X,
                    op=mybir.AluOpType.max, apply_absolute_value=True,
                )
                sxc = spool.tile([128, 1], F32, tag="sxc")
                nc.vector.tensor_scalar(
                    sxc[:], sx[:], EPS, None, op0=mybir.AluOpType.max,
                )
                xs_t = spool.tile([128, 1], F32, tag="xs_t")  # x_scale
                nc.vector.tensor_scalar(
                    xs_t[:], sxc[:], 1.0 / QMAX, None, op0=mybir.AluOpType.mult,
                )
                ix = newton_recip(spool, xs_t, "ix")
                xsws = spool.tile([128, 1], F32, tag="xsws")
                nc.vector.tensor_tensor(
                    xsws[:], xs_t[:], ws_t[:], op=mybir.AluOpType.mult,
                )
                xt2 = bpool.tile([128, DIN], F32, tag="wt2", name="xt2", bufs=1)
                nc.scalar.activation(
                    xt2[:], xt[:], mybir.ActivationFunctionType.Copy,
                    bias=MAGIC, scale=ix[:],
                )
                xq_t = xpool.tile([128, DIN], BF16, tag="xq_t", bufs=1)
                nc.scalar.activation(
                    xq_t[:], xt2[:], mybir.ActivationFunctionType.Copy, bias=-MAGIC,
                )
                xqT = xpool.tile([128, KT, 128], BF16, tag="xqT")
                nc.sync.dma_start(xqT[:], xq_t[:], transpose=True)

                # ---- matmuls ----
                psum_b = ppool.tile([128, DOUT_C], F32, tag="psum_b")
                psum_xa = p2pool.tile([128, RANK], F32, tag="psum_xa")
                for k in range(KT):
                    lhs = xqT[:, k, :]
                    for c in range(NCH):
                        nc.tensor.matmul(
                            psum_b[:, c * 512:(c + 1) * 512],
                            lhs, wqT[c][:, k, :],
                            start=(k == 0), stop=False,
                        )
                    nc.tensor.matmul(
                        psum_xa[:], lhs, a_q[:, k, :],
                        start=(k == 0), stop=(k == KT - 1),
                    )

                # ---- lora second stage: split xa*c2 into 2 bf16 pieces ----
                v_xa = spool.tile([128, RANK], F32, tag="v_xa")
                nc.vector.tensor_scalar(
                    v_xa[:], psum_xa[:], c2[:], None, op0=mybir.AluOpType.mult,
                )
                # hi at cols 0:16, lo at cols 32:48 -> after transpose the
                # pieces sit at 32-aligned base partitions (BIR requires
                # partition access to start at 0/32/64/96).
                pieces = spool.tile([128, 4 * RANK], BF16, tag="pieces")
                nc.vector.tensor_copy(pieces[:, 0:RANK], v_xa[:])
                hi_f = spool.tile([128, RANK], F32, tag="hi_f")
                nc.vector.tensor_copy(hi_f[:], pieces[:, 0:RANK])
                nc.vector.tensor_tensor(
                    pieces[:, 2 * RANK:3 * RANK], v_xa[:], hi_f[:],
                    op=mybir.AluOpType.subtract,
                )
                piecesT_ps = p2pool.tile([4 * RANK, 128], BF16, tag="piecesT_ps")
                nc.tensor.transpose(piecesT_ps[:], pieces[:], ident[:])
                # each piece copied to a base-0 tile (PE needs matching
                # base partitions for lhsT and rhs)
                piecesT = [
                    spool.tile([RANK, 128], BF16, tag=f"piecesT{p}",
                               name=f"piecesT{p}")
                    for p in range(2)
                ]
                for p in range(2):
                    nc.scalar.copy(
                        piecesT[p][:],
                        piecesT_ps[2 * p * RANK:(2 * p + 1) * RANK, :])
                for p in range(2):
                    lhs_p = piecesT[p][:]
                    for c in range(NCH):
                        nc.tensor.matmul(
                            psum_b[:, c * 512:(c + 1) * 512],
                            lhs_p, b_q[:, c * 512:(c + 1) * 512],
                            start=False, stop=(p == 1),
                        )

                # ---- epilogue: out = psum * (x_scale*w_scale) + bias ----
                u = xpool.tile([128, DOUT_C], F32, tag="u", bufs=1)
                nc.scalar.activation(
                    u[:], psum_b[:], mybir.ActivationFunctionType.Copy,
                    bias=0.0, scale=xsws[:],
                )
                nc.vector.tensor_tensor(
                    u[:], u[:], bias_b[:], op=mybir.AluOpType.add,
                )
                nc.scalar.dma_start(out_d[m * 128:(m + 1) * 128, :], u[:])

    nc.compile()
    return nc


# ----------------------------------------------------------------------
# host-side wrapper
# ----------------------------------------------------------------------

@functools.lru_cache(maxsize=2)
def _get_nc(TOK, DIN, DOUT_C, WSC_ROWS, N_FULL_W):
    return build_nc(TOK, DIN, DOUT_C, WSC_ROWS, N_FULL_W)


def _prep(x, weight, bias, lora_A, lora_B):
    B, S, DIN = x.shape
    DOUT = weight.shape[0]
    NTOK = B * S
    assert NTOK % R_TOK == 0 and DOUT % C_OUT == 0 and DOUT % N_CORES == 0
    TOK = NTOK // R_TOK
    DOUT_C = DOUT // C_OUT
    WSC_ROWS = DOUT // N_CORES
    N_FULL_W = DOUT * DIN

    nc = _get_nc(TOK, DIN, DOUT_C, WSC_ROWS, N_FULL_W)

    x2 = np.ascontiguousarray(x.reshape(NTOK, DIN).astype(np.float32, copy=False))
    weight = np.ascontiguousarray(weight.astype(np.float32, copy=False))
    ident = np.eye(128, dtype=ml_dtypes.bfloat16)

    in_maps = []
    for core in range(N_CORES):
        i, j = core // C_OUT, core % C_OUT
        in_maps.append({
            "x_in": np.ascontiguousarray(x2[i * TOK:(i + 1) * TOK]),
            "w_in": np.ascontiguousarray(weight[j * DOUT_C:(j + 1) * DOUT_C]),
            "wsc_in": np.ascontiguousarray(
                weight[core * WSC_ROWS:(core + 1) * WSC_ROWS]),
            "bias_in": np.ascontiguousarray(
                bias[j * DOUT_C:(j + 1) * DOUT_C].reshape(1, DOUT_C)),
            "la_in": np.ascontiguousarray(lora_A.astype(np.float32, copy=False)),
            "lbf_in": np.ascontiguousarray(lora_B.astype(np.float32, copy=False)),
            "lb_in": np.ascontiguousarray(lora_B[:, j * DOUT_C:(j + 1) * DOUT_C]),
            "ident_in": ident,
        })
    return nc, in_maps, (B, S, NTOK, TOK, DOUT, DOUT_C)


def kernel(x, weight, bias, lora_A, lora_B):
    from concourse.bass_utils import run_bass_kernel_spmd

    nc, in_maps, (B, S, NTOK, TOK, DOUT, DOUT_C) = _prep(
        x, weight, bias, lora_A, lora_B)
    res = run_bass_kernel_spmd(nc, in_maps, core_ids=list(range(N_CORES)))

    out = np.empty((NTOK, DOUT), np.float32)
    for core in range(N_CORES):
        i, j = core // C_OUT, core % C_OUT
        out[i * TOK:(i + 1) * TOK, j * DOUT_C:(j + 1) * DOUT_C] = \
            res.results[core]["out"]
    return out.reshape(B, S, DOUT)


def _install_profile_shim():
    """Register the axon NTFF profile hook (antenv.axon_hooks is absent in
    this image; libaxon_pjrt.so supports the profile C ABI directly) and
    stub out the network-dependent artifact upload."""
    import types
    import ctypes
    import contextlib

    try:
        import antenv.axon_hooks  # noqa: F401
        have = True
    except ImportError:
        have = False
    if not have:
        so = "/opt/axon/libaxon_pjrt.so"
        lib = ctypes.CDLL(so)
        lib.axon_start_nrt_profile.argtypes = [
            ctypes.POINTER(ctypes.c_int64), ctypes.c_size_t]
        lib.axon_start_nrt_profile.restype = ctypes.c_int64
        lib.axon_stop_nrt_profile.argtypes = [ctypes.c_char_p]
        lib.axon_stop_nrt_profile.restype = ctypes.c_int64

        @contextlib.contextmanager
        def _hook(output_dir, device_ids):
            import jax
            jax.devices()
            if device_ids:
                ids = (ctypes.c_int64 * len(device_ids))(*device_ids)
                rc = lib.axon_start_nrt_profile(ids, len(device_ids))
            else:
                rc = lib.axon_start_nrt_profile(None, 0)
            if rc != 0:
                raise RuntimeError(f"axon_start_nrt_profile rc={rc}")
            try:
                yield
            finally:
                lib.axon_stop_nrt_profile(str(output_dir).encode())

        import antenv
        mod = types.ModuleType("antenv.axon_hooks")
        mod.get_axon_ntff_profile_hook = lambda: _hook
        mod.set_axon_ntff_profile_hook = lambda h: None
        sys.modules["antenv.axon_hooks"] = mod
        antenv.axon_hooks = mod

    from concourse import bass_utils
    bass_utils.upload_artifacts = lambda tmpdir: f"local:{tmpdir}"


def timed_run(inputs, trace_cores=None):
    """Run with NTFF tracing; returns max exec_time_ns across traced cores."""
    import tempfile
    _install_profile_shim()
    from concourse.bass_utils import run_bass_kernel_spmd

    nc, in_maps, _ = _prep(**inputs)
    res = run_bass_kernel_spmd(
        nc, in_maps, core_ids=list(range(N_CORES)), trace=True,
        trace_cores=trace_cores if trace_cores is not None
        else list(range(N_CORES)),
        tmpdir=tempfile.mkdtemp(prefix="dyadic_trace_"),
    )
    return res.exec_time_ns

